# revision 40
# baseline (speedup 1.0000x reference)
"""Trainium2 Bass kernel for the 4-group sparse-tap 3x3 conv.

Computation (see reference): x (32,128,56,56) f32, weights (32,2048) f32.
Four groups of 32 output channels; group g uses 4 taps CFG[g] of the 3x3
footprint over all 128 input channels.  Output (32,128,56,56) f32.

Strategy: pure data-parallel over batch — 4 images per NeuronCore, 8 cores.

Host prep: zero-pad each image to 59x58 (1-pixel conv halo + one extra row
so the last shifted matmul view stays in-bounds), cast to fp16, and lay the
4 images of a core out channel-major ([128 ic, 4*59*58]) so shards DMA with
large contiguous per-partition descriptors.  Weights are rearranged into 16
[ic=128, oc=32] fp16 stationary blocks, one per (group, tap) pair.

Device, per image: for each chunk of output rows, issue 16 column-tiled
matmuls (tile_position=(0,32g)) — group g's 4 taps accumulate into PSUM
partitions 32g..32g+31.  Tap (kr,kc) uses the rhs slice starting at
(r0+kr)*58+kc, which yields all chunk rows in one contiguous view thanks
to the width padding.  (A 2D AP that skips the 2 pad columns per row was
measured SLOWER: the PE pays ~5-7 cycles per AP row boundary.)  The 4
groups' matmuls execute concurrently on the PE's 32-column sub-arrays, so
an 8-row chunk costs 4x464 PE cycles (~775ns warm).  fp16 keeps 10
mantissa bits and accumulates in fp32; outputs are stored fp16 (DVE casts
on the PSUM->SBUF strip copy) and upcast to f32 on the host.

Timeline model (measured over several NTFF traces):
- exec time runs from the framework const-memset to the last teardown
  instruction; the teardown is a fixed ~7.8us all-engine semaphore sweep
  gated on the LAST engine retiring — so compute-end and last-retire are
  the two numbers that matter.
- a HWDGE ring's first dma_start pays ~2.0us before data moves, the 16
  queue-engines arm staggered over ~0.9us, and a piece's completion
  semaphore fires ~0.3-0.8us after the burst (slowest engine).  Ungated
  queued pieces stream back-to-back; a gated piece pays ~0.7us (ring
  busy) to ~1.7us (ring idle) from gate-clear to first byte.
- the PE runs at 1.2GHz until the HAM's free-running ~3.4us activity
  window sees sustained work, then 2.4GHz (~12.5-12.9us wall in good
  runs).  Any PE idle gap before the boost pushes it a full window later
  (measured +2.7 to +7us) — so the dummy-matmul bridge and stall-free
  cold chunks are load-bearing: inputs stay on big early pieces, and
  dummies (128-elem, so the handoff quantizes to ~0.13us) are sized to
  end ~12.2us, past the weights-semaphore clear even on runs where HBM
  runs 40% slow (input rates vary 160-350B/ns run to run).

Compute order is img0, img1, then img3/img2 interleaved (both input on
the gpsimd SWDGE ring; each image has its own SBUF output slot, so there
are no WAR hazards).  The interleave spreads both images' chunk
completions over the last ~12us of compute so their outputs drain in
v_sem-gated pieces DURING compute: out2 rows 0-51 on scalar, out3 rows
24-55 plus the final tiny out2 rows 52-55 on sync — the fastest ring
(280-390B/ns even when the chip throttles, vs the SWDGE's 160-220) and
idle from ~24us.  The SWDGE ring keeps only out3 rows 0-23 and retires
early, so its laggy gated-issue behavior (measured ~2.5-3us re-arm on an
idle ring, ~0.5-1.2us chained) never sits on the critical path.  img2's
final 8 rows are split 4+2+2 chunks so the post-compute flush is one
57KB piece."""

from contextlib import ExitStack

import numpy as np

import concourse.bass as bass
import concourse.mybir as mybir
from concourse.bass_utils import run_bass_kernel_spmd

CFG = [[1, 2, 4, 5], [2, 3, 5, 6], [4, 5, 7, 8], [5, 6, 8, 9]]

B, C, H, W = 32, 128, 56, 56
NCORES = 8
BPC = B // NCORES            # images per core
HP, WP = H + 3, W + 2        # padded rows (1 top, 1 bottom, 1 overread), cols
XF = HP * WP                 # 3422 padded free elems per image
OF = H * W                   # 3136 output free elems per image
NPSUM = 8                    # psum banks cycled over chunks
F32 = mybir.dt.float32
F16 = mybir.dt.float16

N_DUMMY = 40                 # 128-elem warmup matmuls (~128ns each @1.2GHz)
                             # sized to end ~12.2us, past the weights-piece
                             # semaphore clear even on slow-HBM runs — ending
                             # early leaves a pre-boost PE gap that slips the
                             # HAM window (~+2.7us)

# chunk schedule: (image, first output row, rows).  img0, img1, then img3
# and img2 INTERLEAVED: img2's chunks complete gradually over the last
# ~11us of compute, so its output drains during compute instead of
# flushing ~800KB after compute-end; only its final 4 rows (split 2+2)
# land post-compute.
CHUNKS = [(b, 8 * c, 8) for b in (0, 1) for c in range(7)]
CHUNKS += [(3, 0, 8), (3, 8, 8), (3, 16, 8)]
for _c in range(3):
    CHUNKS += [(2, 8 * _c, 8), (3, 8 * (_c + 3), 8)]
CHUNKS += [(2, 24, 8), (3, 48, 8), (2, 32, 8), (2, 40, 8)]
CHUNKS += [(2, 48, 4), (2, 52, 2), (2, 54, 2)]
NCHUNK = len(CHUNKS)         # 30

# v_sem value after image b's chunk starting at row r0 is copied out
_VIDX = {}
for _g, (_b, _r0, _nr) in enumerate(CHUNKS):
    _VIDX[(_b, _r0)] = _g + 1


def _build_nc():
    nc = bass.Bass()
    xp = nc.declare_dram_parameter("x", [C, BPC * XF], F16, isOutput=False)
    wp = nc.declare_dram_parameter("w", [C, 16 * 32], F16, isOutput=False)
    op = nc.declare_dram_parameter("out", [BPC, C, OF], F16, isOutput=True)

    with ExitStack() as ctx:
        w_tile = ctx.enter_context(nc.sbuf_tensor("w_tile", [C, 16 * 32], F16))
        xbuf = ctx.enter_context(nc.sbuf_tensor("xbuf", [C, BPC * XF], F16))
        o_slots = [ctx.enter_context(nc.sbuf_tensor(f"o_slot{i}", [C, OF], F16))
                   for i in range(BPC)]
        psums = [ctx.enter_context(nc.psum_tensor(f"psum{i}", [C, 512], F32))
                 for i in range(NPSUM)]

        x_sync = ctx.enter_context(nc.semaphore("x_sync"))
        x_sc = ctx.enter_context(nc.semaphore("x_sc"))
        x_gp = ctx.enter_context(nc.semaphore("x_gp"))
        o_sync = ctx.enter_context(nc.semaphore("o_sync"))
        o_sc = ctx.enter_context(nc.semaphore("o_sc"))
        o_gp = ctx.enter_context(nc.semaphore("o_gp"))
        mm_sem = ctx.enter_context(nc.semaphore("mm_sem"))
        v_sem = ctx.enter_context(nc.semaphore("v_sem"))

        block = ctx.enter_context(nc.Block(no_gpsimd_drain=True))

        def img_piece(dma, b, r_lo, r_hi, sem):
            lo, hi = b * XF + r_lo * WP, b * XF + r_hi * WP
            dma.dma_start(out=xbuf[:, lo:hi], in_=xp[:, lo:hi]).then_inc(sem, 16)

        def out_piece(dma, b, row_lo, row_hi, sem):
            # output rows [row_lo, row_hi) of image b
            dma.dma_start(
                out=op[b][:, row_lo * W:row_hi * W],
                in_=o_slots[b][:, row_lo * W:row_hi * W],
            ).then_inc(sem, 16)

        def wait_out(dma, b, row_lo):
            # wait until image b's chunk starting at row_lo has been copied
            dma.wait_ge(v_sem, _VIDX[(b, row_lo)])

        @block.sync
        def _(sync):
            # Big early pieces: fine-grained input pieces were measured to
            # arrive LATER (ring-arming stagger + 3-ring contention), and
            # any cold-chunk stall before the HAM boost costs a ~3.4us
            # window slip.
            img_piece(sync, 0, 0, 26, x_sync)       # img0 rows 0-25 (c0-2)
            img_piece(sync, 0, 26, HP, x_sync)      # img0 rows 26-58 (c3-6)
            wait_out(sync, 0, 48)
            out_piece(sync, 0, 0, H, o_sync)        # out0 whole
            # The late drain rides this ring: sync is idle from ~24us and is
            # the fastest ring (280-387B/ns vs the gpsimd SWDGE's 160-220),
            # so out3's tail and the final tiny out2 piece land soonest here.
            wait_out(sync, 3, 32)
            out_piece(sync, 3, 24, 40, o_sync)      # out3 rows 24-39
            wait_out(sync, 3, 48)
            out_piece(sync, 3, 40, H, o_sync)       # out3 rows 40-55
            wait_out(sync, 2, 54)
            out_piece(sync, 2, 52, H, o_sync)       # out2 rows 52-55 (tiny)
            sync.wait_ge(o_sync, 64)

        @block.scalar
        def _(scalar):
            # weights in two j-major halves: chunk 0's j=0/1 matmuls need
            # only the first 64KB, whose completion (incl. the slowest
            # queue-engine) lands strictly earlier than a single 128KB piece.
            scalar.dma_start(out=w_tile[:, 0:256],
                             in_=wp[:, 0:256]).then_inc(x_sc, 16)
            scalar.dma_start(out=w_tile[:, 256:512],
                             in_=wp[:, 256:512]).then_inc(x_sc, 16)
            scalar.wait_ge(x_sync, 16)              # let img0-front go first:
            img_piece(scalar, 1, 0, HP, x_sc)       # an ungated img1 was
            wait_out(scalar, 1, 48)                 # measured to add +-1us
            out_piece(scalar, 1, 0, H, o_sc)        # variance to img0-front,
            wait_out(scalar, 2, 16)                 # risking a boost slip
            out_piece(scalar, 2, 0, 24, o_sc)       # out2 rows 0-23
            wait_out(scalar, 2, 32)
            out_piece(scalar, 2, 24, 40, o_sc)      # out2 rows 24-39
            wait_out(scalar, 2, 48)
            out_piece(scalar, 2, 40, 52, o_sc)      # out2 rows 40-51
            scalar.wait_ge(o_sc, 64)

        @block.gpsimd
        def _(gpsimd):
            gpsimd.wait_ge(x_sync, 32)              # keep HBM for img0/img1
            img_piece(gpsimd, 3, 0, HP, x_gp)       # img3 whole (computed 3rd)
            img_piece(gpsimd, 2, 0, HP, x_gp)       # img2 whole (computed last)
            # Only out3's early rows stay on the slow SWDGE ring; it then
            # retires long before compute ends and its laggy transfers
            # never sit on the critical path.
            wait_out(gpsimd, 3, 16)
            out_piece(gpsimd, 3, 0, 24, o_gp)       # out3 rows 0-23
            gpsimd.wait_ge(o_gp, 16)

        @block.tensor
        def _(tensor):
            # dummy matmuls on garbage data: continuous PE activity from the
            # earliest possible moment keeps the HAM boost window alive; any
            # pre-boost idle gap delays the 1.2->2.4GHz transition by a full
            # ~3.4us window (measured).  128-elem streams quantize the
            # dummy->real handoff to ~0.13us.
            for _ in range(N_DUMMY):
                tensor.matmul(
                    psums[NPSUM - 1][0:32, :128],
                    w_tile[:, 0:32],
                    xbuf[:, 0:128],
                    start=True, stop=True,
                    tile_position=(0, 0),
                )
            tensor.wait_ge(x_sc, 16)        # weights first half (taps j=0,1)
            tensor.wait_ge(x_sync, 16)      # img0 rows 0-25
            # (global chunk -> input-piece semaphore threshold) gates
            gates = {3: (x_sync, 32), 7: (x_sc, 48),
                     14: (x_gp, 16), 17: (x_gp, 32)}
            for g, (b, r0, nr) in enumerate(CHUNKS):
                if g in gates:
                    tensor.wait_ge(*gates[g])
                if g >= NPSUM:
                    # psum bank g%NPSUM free once chunk g-NPSUM was copied
                    tensor.wait_ge(v_sem, g - NPSUM + 1)
                bank = psums[g % NPSUM]
                nfree = nr * WP
                for j in range(4):
                    if g == 0 and j == 2:
                        tensor.wait_ge(x_sc, 32)    # weights second half
                    for grp in range(4):
                        t = CFG[grp][j]
                        kr, kc = (t - 1) // 3, (t - 1) % 3
                        off = b * XF + (r0 + kr) * WP + kc
                        idx = j * 4 + grp
                        mm = tensor.matmul(
                            bank[32 * grp:32 * (grp + 1), :nfree],
                            w_tile[:, idx * 32:(idx + 1) * 32],
                            xbuf[:, off:off + nfree],
                            start=(j == 0),
                            stop=(j == 3),
                            tile_position=(0, 32 * grp),
                        )
                mm.then_inc(mm_sem, 1)

        @block.vector
        def _(vector):
            for g, (b, r0, nr) in enumerate(CHUNKS):
                vector.wait_ge(mm_sem, g + 1)
                src = psums[g % NPSUM][:, :nr * WP].rearrange(
                    "p (r w) -> p r w", w=WP)[:, :, :W]
                dst = o_slots[b][:, r0 * W:(r0 + nr) * W].rearrange(
                    "p (r w) -> p r w", w=W)
                vector.tensor_copy(out=dst, in_=src).then_inc(v_sem, 1)

    return nc


_NC_CACHE = None


def _get_nc():
    global _NC_CACHE
    if _NC_CACHE is None:
        _NC_CACHE = _build_nc()
    return _NC_CACHE


def _prep_weights(weights):
    """(32, 2048) grouped-sparse -> 16 [ic=128, oc=32] fp16 lhsT blocks,
    ordered j-major so the first DMA half covers tap steps j=0,1."""
    w16 = np.zeros((C, 16 * 32), np.float32)
    for g, taps in enumerate(CFG):
        blk = np.asarray(weights[:, g * 512:(g + 1) * 512], np.float32)
        blk = blk.reshape(32, C, 4)  # [oc_in_group, ic, tap_j]
        for j in range(4):
            idx = j * 4 + g
            w16[:, idx * 32:(idx + 1) * 32] = blk[:, :, j].T
    return np.ascontiguousarray(w16.astype(np.float16))


def _prep_x(x):
    """(32,128,56,56) f32 -> per-core channel-major padded fp16 shards."""
    xpad = np.zeros((B, C, HP, WP), np.float16)
    xpad[:, :, 1:H + 1, 1:W + 1] = x.astype(np.float16)
    xs = xpad.reshape(NCORES, BPC, C, XF)
    # (core, b, c, f) -> (core, c, b*f)
    xs = np.ascontiguousarray(xs.transpose(0, 2, 1, 3)).reshape(NCORES, C, BPC * XF)
    return xs


def _in_maps(x, weights):
    xs = _prep_x(x)
    wflat = _prep_weights(weights)
    return [{"x": xs[i], "w": wflat} for i in range(NCORES)]


def kernel(x, weights):
    x = np.asarray(x, np.float32)
    weights = np.asarray(weights, np.float32)

    nc = _get_nc()
    in_maps = _in_maps(x, weights)
    res = run_bass_kernel_spmd(nc, in_maps, core_ids=list(range(NCORES)))
    return np.concatenate(
        [res.results[i]["out"].astype(np.float32).reshape(BPC, C, H, W)
         for i in range(NCORES)],
        axis=0,
    )


# revision 42
# speedup vs baseline: 1.1334x; 1.1334x over previous
"""Trainium2 Bass kernel for the 4-group sparse-tap 3x3 conv.

Computation (see reference): x (32,128,56,56) f32, weights (32,2048) f32.
Four groups of 32 output channels; group g uses 4 taps CFG[g] of the 3x3
footprint over all 128 input channels.  Output (32,128,56,56) f32.

Strategy: pure data-parallel over batch — 4 images per NeuronCore, 8 cores.

Host prep: zero-pad each image to 59x58 (1-pixel conv halo + one extra row
so the last shifted matmul view stays in-bounds), cast to fp16, and lay the
4 images of a core out channel-major ([128 ic, 4*59*58]) so shards DMA with
large contiguous per-partition descriptors.  Weights are rearranged into 16
[ic=128, oc=32] fp16 stationary blocks, one per (group, tap) pair.

Device, per image: for each chunk of output rows, issue 16 column-tiled
matmuls (tile_position=(0,32g)) — group g's 4 taps accumulate into PSUM
partitions 32g..32g+31.  Tap (kr,kc) uses the rhs slice starting at
(r0+kr)*58+kc, which yields all chunk rows in one contiguous view thanks
to the width padding.  (A 2D AP that skips the 2 pad columns per row was
measured SLOWER: the PE pays ~5-7 cycles per AP row boundary.)  The 4
groups' matmuls execute concurrently on the PE's 32-column sub-arrays, so
an 8-row chunk costs 4x464 PE cycles (~775ns warm).  fp16 keeps 10
mantissa bits and accumulates in fp32; outputs are stored fp16 (DVE casts
on the PSUM->SBUF strip copy) and upcast to f32 on the host.

Timeline model (measured over several NTFF traces):
- exec time runs from the framework const-memset to the last teardown
  instruction; the teardown is a fixed ~7.8us all-engine semaphore sweep
  gated on the LAST engine retiring — so compute-end and last-retire are
  the two numbers that matter.
- a HWDGE ring's first dma_start pays ~2.0us before data moves, the 16
  queue-engines arm staggered over ~0.9us, and a piece's completion
  semaphore fires ~0.3-0.8us after the burst (slowest engine).  Ungated
  queued pieces stream back-to-back; a gated piece pays ~0.7us (ring
  busy) to ~1.7us (ring idle) from gate-clear to first byte.
- the PE runs at 1.2GHz until the HAM's free-running ~3.4us activity
  window sees sustained work, then 2.4GHz (~12.5-12.9us wall in good
  runs).  Any PE idle gap before the boost pushes it a full window later
  (measured +2.7 to +7us) — so the dummy-matmul bridge and stall-free
  cold chunks are load-bearing: inputs stay on big early pieces, and
  dummies (128-elem, so the handoff quantizes to ~0.13us) are sized to
  end ~12.2us, past the weights-semaphore clear even on runs where HBM
  runs 40% slow (input rates vary 160-350B/ns run to run).

Compute order is img0, img1, then img3/img2 interleaved (both input on
the gpsimd SWDGE ring; each image has its own SBUF output slot, so there
are no WAR hazards).  The interleave spreads both images' chunk
completions over the last ~12us of compute so their outputs drain in
v_sem-gated pieces DURING compute: out2 rows 0-51 on scalar, out3 rows
24-55 plus the final tiny out2 rows 52-55 on sync — the fastest ring
(280-390B/ns even when the chip throttles, vs the SWDGE's 160-220) and
idle from ~24us.  The SWDGE ring keeps only out3 rows 0-23 and retires
early, so its laggy gated-issue behavior (measured ~2.5-3us re-arm on an
idle ring, ~0.5-1.2us chained) never sits on the critical path.  img2's
final 8 rows are split 4+2+2 chunks so the post-compute flush is one
57KB piece."""

from contextlib import ExitStack

import numpy as np

import concourse.bass as bass
import concourse.mybir as mybir
from concourse.bass_utils import run_bass_kernel_spmd

CFG = [[1, 2, 4, 5], [2, 3, 5, 6], [4, 5, 7, 8], [5, 6, 8, 9]]

B, C, H, W = 32, 128, 56, 56
NCORES = 8
BPC = B // NCORES            # images per core
HP, WP = H + 3, W + 2        # padded rows (1 top, 1 bottom, 1 overread), cols
XF = HP * WP                 # 3422 padded free elems per image
OF = H * W                   # 3136 output free elems per image
NPSUM = 8                    # psum banks cycled over chunks
F32 = mybir.dt.float32
F16 = mybir.dt.float16

N_DUMMY = 40                 # 128-elem warmup matmuls (~128ns each @1.2GHz)
                             # sized to end ~12.2us, past the weights-piece
                             # semaphore clear even on slow-HBM runs — ending
                             # early leaves a pre-boost PE gap that slips the
                             # HAM window (~+2.7us)

# chunk schedule: (image, first output row, rows).  img0, img1, then img3
# and img2 INTERLEAVED: img2's chunks complete gradually over the last
# ~11us of compute, so its output drains during compute instead of
# flushing ~800KB after compute-end; only its final 4 rows (split 2+2)
# land post-compute.
CHUNKS = [(b, 8 * c, 8) for b in (0, 1) for c in range(7)]
CHUNKS += [(3, 0, 8), (3, 8, 8), (3, 16, 8)]
for _c in range(3):
    CHUNKS += [(2, 8 * _c, 8), (3, 8 * (_c + 3), 8)]
CHUNKS += [(2, 24, 8), (3, 48, 8), (2, 32, 8), (2, 40, 8)]
CHUNKS += [(2, 48, 4), (2, 52, 2), (2, 54, 2)]
NCHUNK = len(CHUNKS)         # 30

# v_sem value after image b's chunk starting at row r0 is copied out
_VIDX = {}
for _g, (_b, _r0, _nr) in enumerate(CHUNKS):
    _VIDX[(_b, _r0)] = _g + 1


def _build_nc():
    nc = bass.Bass()
    xp = nc.declare_dram_parameter("x", [C, BPC * XF], F16, isOutput=False)
    wp = nc.declare_dram_parameter("w", [C, 16 * 32], F16, isOutput=False)
    op = nc.declare_dram_parameter("out", [BPC, C, OF], F16, isOutput=True)

    with ExitStack() as ctx:
        w_tile = ctx.enter_context(nc.sbuf_tensor("w_tile", [C, 16 * 32], F16))
        xbuf = ctx.enter_context(nc.sbuf_tensor("xbuf", [C, BPC * XF], F16))
        o_slots = [ctx.enter_context(nc.sbuf_tensor(f"o_slot{i}", [C, OF], F16))
                   for i in range(BPC)]
        psums = [ctx.enter_context(nc.psum_tensor(f"psum{i}", [C, 512], F32))
                 for i in range(NPSUM)]

        x_sync = ctx.enter_context(nc.semaphore("x_sync"))
        x_sc = ctx.enter_context(nc.semaphore("x_sc"))
        x_gp = ctx.enter_context(nc.semaphore("x_gp"))
        o_sync = ctx.enter_context(nc.semaphore("o_sync"))
        o_sc = ctx.enter_context(nc.semaphore("o_sc"))
        o_gp = ctx.enter_context(nc.semaphore("o_gp"))
        mm_sem = ctx.enter_context(nc.semaphore("mm_sem"))
        v_sem = ctx.enter_context(nc.semaphore("v_sem"))

        block = ctx.enter_context(nc.Block(no_gpsimd_drain=True))

        def img_piece(dma, b, r_lo, r_hi, sem):
            lo, hi = b * XF + r_lo * WP, b * XF + r_hi * WP
            dma.dma_start(out=xbuf[:, lo:hi], in_=xp[:, lo:hi]).then_inc(sem, 16)

        def out_piece(dma, b, row_lo, row_hi, sem):
            # output rows [row_lo, row_hi) of image b
            dma.dma_start(
                out=op[b][:, row_lo * W:row_hi * W],
                in_=o_slots[b][:, row_lo * W:row_hi * W],
            ).then_inc(sem, 16)

        def wait_out(dma, b, row_lo):
            # wait until image b's chunk starting at row_lo has been copied
            dma.wait_ge(v_sem, _VIDX[(b, row_lo)])

        @block.sync
        def _(sync):
            # Big early pieces: fine-grained input pieces were measured to
            # arrive LATER (ring-arming stagger + 3-ring contention), and
            # any cold-chunk stall before the HAM boost costs a ~3.4us
            # window slip.
            img_piece(sync, 0, 0, 26, x_sync)       # img0 rows 0-25 (c0-2)
            img_piece(sync, 0, 26, HP, x_sync)      # img0 rows 26-58 (c3-6)
            wait_out(sync, 0, 48)
            out_piece(sync, 0, 0, H, o_sync)        # out0 whole
            # The late drain rides this ring: sync is idle from ~24us and is
            # the fastest ring (280-387B/ns vs the gpsimd SWDGE's 160-220),
            # so out3's tail and the final tiny out2 piece land soonest here.
            # out3 rows 24-55 as ONE piece: its ~1.5us transfer is still in
            # flight when the final piece's gate clears, so the final piece
            # chains behind it instead of paying the measured ~1.4us
            # idle-ring DGE re-arm.
            wait_out(sync, 3, 48)
            out_piece(sync, 3, 24, H, o_sync)       # out3 rows 24-55
            wait_out(sync, 2, 54)
            out_piece(sync, 2, 52, H, o_sync)       # out2 rows 52-55 (tiny)
            sync.wait_ge(o_sync, 48)

        @block.scalar
        def _(scalar):
            scalar.dma_start(out=w_tile[:], in_=wp[:]).then_inc(x_sc, 16)
            scalar.wait_ge(x_sync, 16)              # let img0-front go first:
            img_piece(scalar, 1, 0, HP, x_sc)       # an ungated img1 was
            wait_out(scalar, 1, 48)                 # measured to add +-1us
            out_piece(scalar, 1, 0, H, o_sc)        # variance to img0-front,
            wait_out(scalar, 2, 16)                 # risking a boost slip
            out_piece(scalar, 2, 0, 24, o_sc)       # out2 rows 0-23
            wait_out(scalar, 2, 32)
            out_piece(scalar, 2, 24, 40, o_sc)      # out2 rows 24-39
            wait_out(scalar, 2, 48)
            out_piece(scalar, 2, 40, 52, o_sc)      # out2 rows 40-51
            scalar.wait_ge(o_sc, 64)

        @block.gpsimd
        def _(gpsimd):
            gpsimd.wait_ge(x_sync, 32)              # keep HBM for img0/img1
            img_piece(gpsimd, 3, 0, HP, x_gp)       # img3 whole (computed 3rd)
            img_piece(gpsimd, 2, 0, HP, x_gp)       # img2 whole (computed last)
            # Only out3's early rows stay on the slow SWDGE ring; it then
            # retires long before compute ends and its laggy transfers
            # never sit on the critical path.
            wait_out(gpsimd, 3, 16)
            out_piece(gpsimd, 3, 0, 24, o_gp)       # out3 rows 0-23
            gpsimd.wait_ge(o_gp, 16)

        @block.tensor
        def _(tensor):
            # dummy matmuls on garbage data: continuous PE activity from the
            # earliest possible moment keeps the HAM boost window alive; any
            # pre-boost idle gap delays the 1.2->2.4GHz transition by a full
            # ~3.4us window (measured).  128-elem streams quantize the
            # dummy->real handoff to ~0.13us.
            for _ in range(N_DUMMY):
                tensor.matmul(
                    psums[NPSUM - 1][0:32, :128],
                    w_tile[:, 0:32],
                    xbuf[:, 0:128],
                    start=True, stop=True,
                    tile_position=(0, 0),
                )
            tensor.wait_ge(x_sc, 16)        # weights
            tensor.wait_ge(x_sync, 16)      # img0 rows 0-25
            # (global chunk -> input-piece semaphore threshold) gates
            gates = {3: (x_sync, 32), 7: (x_sc, 32),
                     14: (x_gp, 16), 17: (x_gp, 32)}
            for g, (b, r0, nr) in enumerate(CHUNKS):
                if g in gates:
                    tensor.wait_ge(*gates[g])
                if g >= NPSUM:
                    # psum bank g%NPSUM free once chunk g-NPSUM was copied
                    tensor.wait_ge(v_sem, g - NPSUM + 1)
                bank = psums[g % NPSUM]
                nfree = nr * WP
                for j in range(4):
                    for grp in range(4):
                        t = CFG[grp][j]
                        kr, kc = (t - 1) // 3, (t - 1) % 3
                        off = b * XF + (r0 + kr) * WP + kc
                        idx = grp * 4 + j
                        mm = tensor.matmul(
                            bank[32 * grp:32 * (grp + 1), :nfree],
                            w_tile[:, idx * 32:(idx + 1) * 32],
                            xbuf[:, off:off + nfree],
                            start=(j == 0),
                            stop=(j == 3),
                            tile_position=(0, 32 * grp),
                        )
                mm.then_inc(mm_sem, 1)

        @block.vector
        def _(vector):
            for g, (b, r0, nr) in enumerate(CHUNKS):
                vector.wait_ge(mm_sem, g + 1)
                src = psums[g % NPSUM][:, :nr * WP].rearrange(
                    "p (r w) -> p r w", w=WP)[:, :, :W]
                dst = o_slots[b][:, r0 * W:(r0 + nr) * W].rearrange(
                    "p (r w) -> p r w", w=W)
                vector.tensor_copy(out=dst, in_=src).then_inc(v_sem, 1)

    return nc


_NC_CACHE = None


def _get_nc():
    global _NC_CACHE
    if _NC_CACHE is None:
        _NC_CACHE = _build_nc()
    return _NC_CACHE


def _prep_weights(weights):
    """(32, 2048) grouped-sparse -> 16 [ic=128, oc=32] fp16 lhsT blocks."""
    w16 = np.zeros((C, 16 * 32), np.float32)
    for g, taps in enumerate(CFG):
        blk = np.asarray(weights[:, g * 512:(g + 1) * 512], np.float32)
        blk = blk.reshape(32, C, 4)  # [oc_in_group, ic, tap_j]
        for j in range(4):
            idx = g * 4 + j
            w16[:, idx * 32:(idx + 1) * 32] = blk[:, :, j].T
    return np.ascontiguousarray(w16.astype(np.float16))


def _prep_x(x):
    """(32,128,56,56) f32 -> per-core channel-major padded fp16 shards."""
    xpad = np.zeros((B, C, HP, WP), np.float16)
    xpad[:, :, 1:H + 1, 1:W + 1] = x.astype(np.float16)
    xs = xpad.reshape(NCORES, BPC, C, XF)
    # (core, b, c, f) -> (core, c, b*f)
    xs = np.ascontiguousarray(xs.transpose(0, 2, 1, 3)).reshape(NCORES, C, BPC * XF)
    return xs


def _in_maps(x, weights):
    xs = _prep_x(x)
    wflat = _prep_weights(weights)
    return [{"x": xs[i], "w": wflat} for i in range(NCORES)]


def kernel(x, weights):
    x = np.asarray(x, np.float32)
    weights = np.asarray(weights, np.float32)

    nc = _get_nc()
    in_maps = _in_maps(x, weights)
    res = run_bass_kernel_spmd(nc, in_maps, core_ids=list(range(NCORES)))
    return np.concatenate(
        [res.results[i]["out"].astype(np.float32).reshape(BPC, C, H, W)
         for i in range(NCORES)],
        axis=0,
    )
